# revision 13
# baseline (speedup 1.0000x reference)
"""Trainium2 Bass kernel for nn_DecodingLoss_BCEBased (segment_reduce).

Strategy v2 (4 batch-groups x 2 check-halves over 8 NeuronCores):
  - Each core covers 256 batch rows (two 128-row blocks j=0,1 packed into
    one 512B token row) and half the checks (5120 incl. pad).
  - Token table in SBUF: tokT[p, stripe*256 + j*128 + b] = tanh(0.5*llr),
    bf16, so each gather descriptor moves 512B (vs 256B at 128 batch/core)
    -> half the descriptors for the same bytes.
  - dma_gather's descriptor generation runs on ONE Q7 core-pair selected by
    queue_num (ucode: cpu_id/2 == queue_num). The v1 kernel put all gathers
    on queue 0, serializing ~38us/chunk on cores 0+1 while cores 2-7 idled;
    that cadence was the whole 750us wall time. v2 builds with
    num_swdge_queues=4 and rotates gathers across queues 0-3 so four
    core-pairs generate descriptors concurrently.
  - Gather idx order is slot-major per chunk (slot s of all 512 checks,
    then slot s+1 ...) so the product-of-8 tree is three unit-stride
    contiguous bf16 multiplies (v1's strided 0::2/1::2 reads ran ~2.5x
    slower than contiguous).
  - BCEWithLogits identity: softplus(z) - z*y with z = -2*arctanh(p)
    equals log2 - log(1 - s*p), s = 2y-1. Per check: tree product, * sgn
    (bf16), clamp <= 1-2^-8, one ACT Ln(1-x) per j with accum_out doing
    the sum-over-checks reduction.
  - Observables (8 x 200, padded to 256 with an all-ones token) use the
    same path on every core; half-0 cores get sgn_obs=0 so they contribute
    nothing (keeps the 8 cores' work identical).
  - Each core returns S[p, j] = sum ln(1-s*p) over its check half; host:
    loss = 0.5*(M+K)*log2 - 0.5*mean_b(S_b).
"""
import numpy as np
import ml_dtypes
import concourse.bass as bass
import concourse.tile as tile
from concourse import bacc, mybir
from concourse.bass_utils import run_bass_kernel_spmd

F32 = mybir.dt.float32
BF16 = mybir.dt.bfloat16
I16 = mybir.dt.int16
AF = mybir.ActivationFunctionType
ALU = mybir.AluOpType
BF = ml_dtypes.bfloat16

P = 128            # SBUF partitions
N_CORES = 8
B, N, M, K = 1024, 20000, 10000, 8
CHK_W, OBS_W = 8, 200

NBG = 4            # batch groups (256 rows each)
NJ = 2             # 128-row blocks per core
BW = NJ * P        # batch rows per core = 256
HALF = M // 2      # checks per half (5000)
CHK_CHUNK = 512
N_CHK_HALF = 5120  # padded checks per core (10 chunks)
N_CHUNKS = N_CHK_HALF // CHK_CHUNK
OBS_PW = 256       # obs support padded to pow2

N_STRIPE = (N + P - 1) // P          # 157 data stripes
ONES_ID = N_STRIPE * P               # token in the all-ones stripe
N_TOK_PAD = N_STRIPE * P             # 20096 (dram rows; ones stripe is SBUF-only)
TOK_ELEMS = (N_STRIPE + 1) * BW      # table free elems per partition (bf16)

GIDX = CHK_CHUNK * CHK_W             # 4096 idx per chunk gather
N_OBS_IDX = K * OBS_PW               # 2048

KMAX = 1.0 - 2.0 ** -8               # clamp, exactly representable in bf16

_NC_CACHE = {}
_TRACE = False  # test.py flips this to get neuron-profile exec_time_ns


def _build_kernel():
    nc = bacc.Bacc("TRN2", target_bir_lowering=False, debug=False,
                   num_devices=N_CORES, num_swdge_queues=4)

    llrsT2 = nc.dram_tensor("llrsT2", [N_TOK_PAD, BW], BF16,
                            kind="ExternalInput").ap()
    sgn = nc.dram_tensor("sgn", [P, NJ * N_CHK_HALF], BF16,
                         kind="ExternalInput").ap()
    sgn_obs = nc.dram_tensor("sgn_obs", [P, NJ * K], BF16,
                             kind="ExternalInput").ap()
    chk_idx = nc.dram_tensor(
        "chk_idx", [P, N_CHK_HALF * CHK_W // 16], I16, kind="ExternalInput").ap()
    obs_idx = nc.dram_tensor(
        "obs_idx", [P, N_OBS_IDX // 16], I16, kind="ExternalInput").ap()
    out = nc.dram_tensor("out", [P, NJ], F32, kind="ExternalOutput").ap()

    with tile.TileContext(nc) as tc:
        with (
            tc.tile_pool(name="tok", bufs=1) as tok_pool,
            tc.tile_pool(name="stage", bufs=2) as stage_pool,
            tc.tile_pool(name="idx", bufs=1) as idx_pool,
            tc.tile_pool(name="g", bufs=4) as g_pool,
            tc.tile_pool(name="gob", bufs=1) as gob_pool,
            tc.tile_pool(name="tree", bufs=1) as tree_pool,
            tc.tile_pool(name="sg", bufs=2) as sg_pool,
            tc.tile_pool(name="spc", bufs=2) as spc_pool,
            tc.tile_pool(name="acc", bufs=1) as acc_pool,
        ):
            chk_idx_t = idx_pool.tile([P, N_CHK_HALF * CHK_W // 16], I16,
                                      tag="ichk")
            nc.sync.dma_start(chk_idx_t[:], chk_idx)
            obs_idx_t = idx_pool.tile([P, N_OBS_IDX // 16], I16, tag="iobs")
            nc.sync.dma_start(obs_idx_t[:], obs_idx)
            sgo = idx_pool.tile([P, NJ * K], BF16, tag="sgo")
            nc.sync.dma_start(sgo[:], sgn_obs)

            acc = acc_pool.tile([P, NJ * (N_CHUNKS + 1)], F32, tag="acc")
            kmax = acc_pool.tile([P, NJ * CHK_CHUNK], BF16, tag="kmax")
            nc.vector.memset(kmax[:], KMAX)

            tokT = tok_pool.tile([P, TOK_ELEMS], BF16)

            def gather(dst3d, idxs_ap, n_idx, q, prep_sem=None):
                return nc.gpsimd.dma_gather(
                    out_ap=dst3d,
                    in_ap=tokT[:],
                    idxs_ap=idxs_ap,
                    num_idxs=n_idx,
                    num_idxs_reg=n_idx,
                    elem_size=BW,            # 256 bf16 = 512B per idx
                    transpose=True,
                    single_packet=False,
                    sbuf_tokens_per_rank=P,
                    sbuf_free_dim_per_rank=BW * 2,
                    sbuf_free_dim_pad_per_rank=0,
                    sbuf_byte_offset=0,
                    queue_num=q,
                    prepare_only=prep_sem is not None,
                    sem=prep_sem,
                )

            # token table: t = tanh(0.5*llrs), bf16, 512B per token row
            r = 0
            while r < N_STRIPE:
                ns = min(8, N_STRIPE - r)
                st = stage_pool.tile([P, 8 * BW], BF16, tag="stage")
                src = llrsT2[bass.ds(r * P, ns * P), :].rearrange(
                    "(rr p) b -> p rr b", p=P)
                dst = st[:, : ns * BW].rearrange("p (rr b) -> p rr b", b=BW)
                nc.sync.dma_start(dst, src)
                nc.scalar.activation(
                    tokT[:, bass.ds(r * BW, ns * BW)], st[:, : ns * BW],
                    AF.Tanh, scale=0.5)
                r += ns
            # ones stripe for obs padding
            nc.vector.memset(tokT[:, bass.ds(N_STRIPE * BW, BW)], 1.0)

            # Prep (descriptor-gen only) the obs gather and the first chunk
            # gather of each queue. The preps' only semaphore waits are the
            # idx tiles, so the Q7 pairs generate descriptors while the token
            # table is still loading; the tokT RAW edge is deferred to the
            # trigger_dma calls, which fire the DMAs once the table is done.
            obs_sem = nc.alloc_semaphore("gs_obs")
            gob = gob_pool.tile([P, NJ * N_OBS_IDX], BF16, tag="gob")
            gather(gob[:].rearrange("p (j i) -> p j i", j=NJ),
                   obs_idx_t[:], N_OBS_IDX, 3, prep_sem=obs_sem)
            g_tiles = {}
            g_sems = {}
            for c in range(4):
                g = g_pool.tile([P, NJ * GIDX], BF16, tag="g", name=f"g{c}")
                g_tiles[c] = g
                g_sems[c] = nc.alloc_semaphore(f"gs{c}")
                gather(g[:].rearrange("p (j i) -> p j i", j=NJ),
                       chk_idx_t[:, bass.ds(c * GIDX // 16, GIDX // 16)],
                       GIDX, c % 4, prep_sem=g_sems[c])

            # fire the prepped gathers the moment the table is complete
            for q in range(4):
                nc.gpsimd.trigger_dma(count=None, queue_num=q)

            # check chunks: slot-major gather -> contiguous mult tree
            for c in range(N_CHUNKS):
                if c in g_tiles:
                    g = g_tiles[c]
                else:
                    g = g_pool.tile([P, NJ * GIDX], BF16, tag="g", name=f"g{c}")
                    gather(g[:].rearrange("p (j i) -> p j i", j=NJ),
                           chk_idx_t[:, bass.ds(c * GIDX // 16, GIDX // 16)],
                           GIDX, c % 4)
                g3 = g[:].rearrange("p (j i) -> p j i", j=NJ)
                p1 = tree_pool.tile([P, NJ * GIDX // 2], BF16, tag="p1")
                p13 = p1[:].rearrange("p (j i) -> p j i", j=NJ)
                l1 = nc.vector.tensor_tensor(p13, g3[:, :, : GIDX // 2],
                                             g3[:, :, GIDX // 2:], ALU.mult)
                if c in g_sems:
                    # prepped gathers: tile's consumer gating fires at
                    # descriptor-write time, so gate the first reader on the
                    # real DMA-completion sem (16 ring increments).
                    l1._wait_ge(g_sems[c], 16)
                p2 = tree_pool.tile([P, NJ * GIDX // 4], BF16, tag="p2")
                p23 = p2[:].rearrange("p (j i) -> p j i", j=NJ)
                nc.vector.tensor_tensor(p23, p13[:, :, : GIDX // 4],
                                        p13[:, :, GIDX // 4:], ALU.mult)
                p3 = tree_pool.tile([P, NJ * CHK_CHUNK], BF16, tag="p3")
                p33 = p3[:].rearrange("p (j i) -> p j i", j=NJ)
                nc.vector.tensor_tensor(p33, p23[:, :, :CHK_CHUNK],
                                        p23[:, :, CHK_CHUNK:], ALU.mult)
                sg = sg_pool.tile([P, NJ * CHK_CHUNK], BF16, tag="sg")
                nc.sync.dma_start(
                    sg[:], sgn[:, bass.ds(c * NJ * CHK_CHUNK, NJ * CHK_CHUNK)])
                sp = sg_pool.tile([P, NJ * CHK_CHUNK], BF16, tag="sp")
                nc.vector.tensor_tensor(sp[:], p3[:], sg[:], ALU.mult)
                spc = spc_pool.tile([P, NJ * CHK_CHUNK], BF16, tag="spc")
                nc.vector.tensor_tensor(spc[:], sp[:], kmax[:], ALU.min)
                lnd = tree_pool.tile([P, NJ * CHK_CHUNK], BF16, tag="lnd")
                for j in range(NJ):
                    nc.scalar.activation(
                        lnd[:, bass.ds(j * CHK_CHUNK, CHK_CHUNK)],
                        spc[:, bass.ds(j * CHK_CHUNK, CHK_CHUNK)],
                        AF.Ln, bias=1.0, scale=-1.0,
                        accum_out=acc[:, bass.ds(c * NJ + j, 1)])

            # observables: slot-major (8 obs contiguous per slot), 256 slots
            cur = gob[:].rearrange("p (j i) -> p j i", j=NJ)
            w = N_OBS_IDX
            lvl = 0
            while w > 2 * K:
                nxt_t = tree_pool.tile([P, NJ * w // 2], BF16, tag=f"ob{lvl}")
                nxt = nxt_t[:].rearrange("p (j i) -> p j i", j=NJ)
                ob_l = nc.vector.tensor_tensor(nxt, cur[:, :, : w // 2],
                                               cur[:, :, w // 2:], ALU.mult)
                if lvl == 0:
                    ob_l._wait_ge(obs_sem, 16)
                cur = nxt
                w //= 2
                lvl += 1
            pob = tree_pool.tile([P, NJ * K], BF16, tag="pob")
            pob3 = pob[:].rearrange("p (j i) -> p j i", j=NJ)
            nc.vector.tensor_tensor(pob3, cur[:, :, :K], cur[:, :, K:],
                                    ALU.mult)
            nc.vector.tensor_tensor(pob[:], pob[:], sgo[:], ALU.mult)
            nc.vector.tensor_tensor(pob[:], pob[:], kmax[:, : NJ * K], ALU.min)
            lno = tree_pool.tile([P, NJ * K], BF16, tag="lno")
            for j in range(NJ):
                nc.scalar.activation(
                    lno[:, bass.ds(j * K, K)], pob[:, bass.ds(j * K, K)],
                    AF.Ln, bias=1.0, scale=-1.0,
                    accum_out=acc[:, bass.ds(NJ * N_CHUNKS + j, 1)])

            s_t = acc_pool.tile([P, NJ], F32, tag="st")
            accv = acc[:].rearrange("p (c j) -> p j c", j=NJ)
            nc.vector.tensor_reduce(s_t[:], accv, mybir.AxisListType.X,
                                    ALU.add)
            nc.sync.dma_start(out, s_t[:])

    nc.compile()
    return nc


def _get_nc():
    if "nc" not in _NC_CACHE:
        _NC_CACHE["nc"] = _build_kernel()
    return _NC_CACHE["nc"]


def _wrap_idx(flat):
    # dma_gather index layout: unwrapped[s*16+p] = tile[p, s], replicated
    # across the eight 16-partition groups
    n = flat.shape[0]
    w = flat.reshape(n // 16, 16).T.astype(np.int16)
    return np.tile(w, (8, 1))


def kernel(llrs, syndromes, observables, chk_cols, obs_cols):
    llrs = np.asarray(llrs, dtype=np.float32)
    syndromes = np.asarray(syndromes, dtype=np.float32)
    observables = np.asarray(observables, dtype=np.float32)
    chk_cols = np.asarray(chk_cols)
    obs_cols = np.asarray(obs_cols)

    nc = _get_nc()

    # token-major llrs, bf16: [N_TOK_PAD, B]
    llrsT = np.zeros((N_TOK_PAD, B), BF)
    llrsT[:N] = llrs.T

    # sgn, padded to N_CHK_HALF per half, laid out [p, c, j, i]
    sgn_full = np.zeros((B, 2 * N_CHK_HALF), BF)
    sgn_full[:, :M] = (2.0 * syndromes - 1.0)
    sgn_obs_full = (2.0 * observables - 1.0).astype(BF)

    # check idx, slot-major per 512-chunk: idx[c*4096 + s*512 + i]
    chk_pad = np.zeros((2 * N_CHK_HALF, CHK_W), np.int64)
    chk_pad[:M] = chk_cols

    def chk_idx_half(h):
        cc = chk_pad[h * N_CHK_HALF:(h + 1) * N_CHK_HALF]
        cc = cc.reshape(N_CHUNKS, CHK_CHUNK, CHK_W).transpose(0, 2, 1)
        return _wrap_idx(cc.reshape(-1))

    chk_idx_w = [chk_idx_half(0), chk_idx_half(1)]

    # obs idx, slot-major: idx[s*8 + k], slots >= 200 -> ones token
    op = np.full((K, OBS_PW), ONES_ID, np.int64)
    op[:, :OBS_W] = obs_cols
    obs_idx_w = _wrap_idx(op.T.reshape(-1))

    in_maps = []
    for core in range(N_CORES):
        bg, half = core // 2, core % 2
        bsl = slice(bg * BW, (bg + 1) * BW)
        # sgn slice -> [p, c, j, i] -> [128, NJ*N_CHK_HALF]
        v = sgn_full[bsl, half * N_CHK_HALF:(half + 1) * N_CHK_HALF]
        v = v.reshape(NJ, P, N_CHUNKS, CHK_CHUNK).transpose(1, 2, 0, 3)
        so = sgn_obs_full[bsl].reshape(NJ, P, K).transpose(1, 0, 2)
        if half == 0:
            so = np.zeros_like(so)
        in_maps.append({
            "llrsT2": np.ascontiguousarray(llrsT[:, bsl]),
            "sgn": np.ascontiguousarray(v.reshape(P, NJ * N_CHK_HALF)),
            "sgn_obs": np.ascontiguousarray(so.reshape(P, NJ * K)),
            "chk_idx": chk_idx_w[half],
            "obs_idx": obs_idx_w,
        })

    res = run_bass_kernel_spmd(nc, in_maps, core_ids=list(range(N_CORES)),
                               trace=_TRACE)
    _NC_CACHE["exec_time_ns"] = res.exec_time_ns
    # S[bg*256 + j*128 + p] = sum over both halves
    S = np.zeros((NBG, NJ, P), np.float64)
    for core in range(N_CORES):
        bg = core // 2
        o = res.results[core]["out"].astype(np.float64)  # [p, j]
        S[bg] += o.T
    S = S.reshape(B)
    loss_b = 0.5 * (M + K) * np.log(2.0) - 0.5 * S
    return np.float32(loss_b.mean())
